# revision 16
# baseline (speedup 1.0000x reference)
"""Trainium2 Bass kernel for nn_Constant2RVLinearlayer (random-variable linear layer).

Computes, for x [B, 128], w_mu [128, 128], w_sigma [128], b_mu [128], b_sigma [128]:
  mu_out    = (x @ w_mu + b_mu)[:, None, :]                    (B, 1, 128)
  sigma_out = (sum_i sp_w[i] * x[:, i]^2)[:, None, None] + sp_b (B, 1, 128)
  kl_loss   = -0.5 * mean(128*log(sp_w) - sum|w_mu| - 128*sp_w)  scalar
with sp_w = softplus(w_sigma), sp_b = softplus(b_sigma).

Sharding: pure data parallel — batch split 8 ways, tiny params replicated.
kl_loss depends only on params and is computed on host in fp32.

Per-core kernel (65536 rows): rows stream through in 2048-row chunks
(1 MiB DMAs). Each 128-row block is transposed on the PE (via identity
matmul) so the contraction dim lands on partitions, then:
  - mu:    PE fp32 matmul (xT stationary, w_mu moving) -> PSUM,
           DVE adds broadcast b_mu while moving PSUM->SBUF.
  - quad:  ACT computes (sqrt(sp_w)*x)^2 from the transposed PSUM tile,
           PE reduces over partitions with a ones-vector matmul.
  - sigma: DVE tensor_scalar add of per-row quad onto broadcast sp_b.
"""

import os

import numpy as np

BATCH = 524288
D = 128
NCORES = 8
ROWS = BATCH // NCORES  # 65536 rows per core
BLK = 128               # rows per partition-tile
MACRO = 4               # 128-row blocks per macro tile (512-wide ops)
CHUNK_BLKS = 16         # 128-row blocks per DMA chunk (2048 rows = 1 MiB)


def _build(
    rows: int,
    out_engine: str = "gpsimd",
    io_bufs: int = 3,
    psum_bufs: int = 3,
    repeat: int = 1,
    mode: str = "full",  # full | dmaonly | dmape (perf-ablation builds)
):
    import contextlib

    import concourse.bacc as bacc
    import concourse.tile as tile
    from concourse import mybir

    f32 = mybir.dt.float32
    AF = mybir.ActivationFunctionType

    n_chunks = rows // (CHUNK_BLKS * BLK)
    assert n_chunks * CHUNK_BLKS * BLK == rows

    nc = bacc.Bacc("TRN2", target_bir_lowering=False, debug=False)

    x_d = nc.dram_tensor("x", [rows, D], f32, kind="ExternalInput")
    wmu_d = nc.dram_tensor("wmu", [D, D], f32, kind="ExternalInput")
    ident_d = nc.dram_tensor("ident", [D, D], f32, kind="ExternalInput")
    bmu4_d = nc.dram_tensor("bmu4", [D, MACRO * BLK], f32, kind="ExternalInput")
    spb_d = nc.dram_tensor("spb", [D, BLK], f32, kind="ExternalInput")
    sqw_d = nc.dram_tensor("sqw", [D, 1], f32, kind="ExternalInput")
    ones_d = nc.dram_tensor("ones", [D, 1], f32, kind="ExternalInput")

    mu_d = nc.dram_tensor("mu", [rows, D], f32, kind="ExternalOutput")
    sg_d = nc.dram_tensor("sigma", [rows, D], f32, kind="ExternalOutput")

    # [rows, D] -> [n, p, j, c]: row = n*2048 + p*16 + j. Each partition's
    # chunk slice is 16 consecutive rows = 8 KiB contiguous in DRAM, so a
    # 1 MiB chunk DMA needs only 128 descriptors. All ops are row-independent,
    # so the within-chunk row permutation is harmless as long as x/mu/sigma
    # use the same mapping.
    xr = x_d.ap().rearrange("(n p j) c -> n p j c", p=BLK, j=CHUNK_BLKS)
    mur = mu_d.ap().rearrange("(n p j) c -> n p j c", p=BLK, j=CHUNK_BLKS)
    sgr = sg_d.ap().rearrange("(n p j) c -> n p j c", p=BLK, j=CHUNK_BLKS)

    with tile.TileContext(nc) as tc:
        out_eng = {
            "sync": nc.sync,
            "scalar": nc.scalar,
            "gpsimd": nc.gpsimd,
        }[out_engine]
        with (
            tc.tile_pool(name="consts", bufs=1) as consts,
            tc.tile_pool(name="xin", bufs=io_bufs) as xin,
            tc.tile_pool(name="xtp", bufs=3) as xtp,
            tc.tile_pool(name="outs", bufs=io_bufs) as outs,
            tc.tile_pool(name="ps_t", bufs=psum_bufs, space="PSUM") as ps_t,
            tc.tile_pool(name="ps_m", bufs=psum_bufs, space="PSUM") as ps_m,
            tc.tile_pool(name="ps_q", bufs=2, space="PSUM") as ps_q,
        ):
            ident = consts.tile([D, D], f32)
            nc.sync.dma_start(out=ident, in_=ident_d.ap())
            wmu = consts.tile([D, D], f32)
            nc.sync.dma_start(out=wmu, in_=wmu_d.ap())
            bmu4 = consts.tile([D, MACRO * BLK], f32)
            nc.sync.dma_start(out=bmu4, in_=bmu4_d.ap())
            spb = consts.tile([D, BLK], f32)
            nc.sync.dma_start(out=spb, in_=spb_d.ap())
            sqw = consts.tile([D, 1], f32)
            nc.sync.dma_start(out=sqw, in_=sqw_d.ap())
            ones = consts.tile([D, 1], f32)
            nc.sync.dma_start(out=ones, in_=ones_d.ap())

            rep_ctx = (
                tc.For_i(0, repeat, 1) if repeat > 1 else contextlib.nullcontext()
            )
            with rep_ctx:
                _emit_body(
                    nc, tc, n_chunks, xr, mur, sgr, xin, xtp, outs,
                    ps_t, ps_m, ps_q, ident, wmu, bmu4, spb, sqw, ones,
                    out_eng, f32, AF, mode,
                )
    nc.compile()
    return nc


def _emit_body(
    nc, tc, n_chunks, xr, mur, sgr, xin, xtp, outs,
    ps_t, ps_m, ps_q, ident, wmu, bmu4, spb, sqw, ones,
    out_eng, f32, AF, mode="full",
):
    if True:
        if True:
            for n in range(n_chunks):
                x_sb = xin.tile([D, CHUNK_BLKS * BLK], f32)
                nc.sync.dma_start(
                    out=x_sb.rearrange("p (j c) -> p j c", c=BLK), in_=xr[n]
                )
                mu_sb = outs.tile([D, CHUNK_BLKS * BLK], f32, tag="mu")
                sg_sb = outs.tile([D, CHUNK_BLKS * BLK], f32, tag="sg")

                if mode == "dmaonly":
                    out_eng.dma_start(
                        out=mur[n], in_=x_sb.rearrange("p (j c) -> p j c", c=BLK)
                    )
                    out_eng.dma_start(
                        out=sgr[n], in_=x_sb.rearrange("p (j c) -> p j c", c=BLK)
                    )
                    continue
                if mode == "dmape":
                    for m in range(CHUNK_BLKS // MACRO):
                        pT = ps_t.tile([D, MACRO * BLK], f32)
                        for q in range(MACRO):
                            j = m * MACRO + q
                            nc.tensor.transpose(
                                out=pT[:, q * BLK : (q + 1) * BLK],
                                in_=x_sb[:, j * BLK : (j + 1) * BLK],
                                identity=ident,
                            )
                        pM = ps_m.tile([D, MACRO * BLK], f32)
                        pQ = ps_q.tile([D, MACRO], f32)
                        for q in range(MACRO):
                            j = m * MACRO + q
                            nc.tensor.matmul(
                                out=pM[:, q * BLK : (q + 1) * BLK],
                                lhsT=x_sb[:, j * BLK : (j + 1) * BLK],
                                rhs=wmu,
                                start=True,
                                stop=True,
                            )
                            nc.tensor.matmul(
                                out=pQ[:, q : q + 1],
                                lhsT=x_sb[:, j * BLK : (j + 1) * BLK],
                                rhs=ones,
                                start=True,
                                stop=True,
                            )
                    out_eng.dma_start(
                        out=mur[n], in_=x_sb.rearrange("p (j c) -> p j c", c=BLK)
                    )
                    out_eng.dma_start(
                        out=sgr[n], in_=x_sb.rearrange("p (j c) -> p j c", c=BLK)
                    )
                    continue

                for m in range(CHUNK_BLKS // MACRO):
                    pT = ps_t.tile([D, MACRO * BLK], f32)
                    for q in range(MACRO):
                        j = m * MACRO + q
                        nc.tensor.transpose(
                            out=pT[:, q * BLK : (q + 1) * BLK],
                            in_=x_sb[:, j * BLK : (j + 1) * BLK],
                            identity=ident,
                        )
                    xT = xtp.tile([D, MACRO * BLK], f32, tag="xT")
                    nc.scalar.copy(out=xT, in_=pT)
                    xsq = xtp.tile([D, MACRO * BLK], f32, tag="xsq")
                    nc.scalar.activation(
                        out=xsq, in_=pT, func=AF.Square, scale=sqw[:, :]
                    )

                    pM = ps_m.tile([D, MACRO * BLK], f32)
                    pQ = ps_q.tile([D, MACRO], f32)
                    for q in range(MACRO):
                        nc.tensor.matmul(
                            out=pM[:, q * BLK : (q + 1) * BLK],
                            lhsT=xT[:, q * BLK : (q + 1) * BLK],
                            rhs=wmu,
                            start=True,
                            stop=True,
                        )
                        nc.tensor.matmul(
                            out=pQ[:, q : q + 1],
                            lhsT=xsq[:, q * BLK : (q + 1) * BLK],
                            rhs=ones,
                            start=True,
                            stop=True,
                        )

                    nc.vector.tensor_add(
                        mu_sb[:, m * MACRO * BLK : (m + 1) * MACRO * BLK], pM, bmu4
                    )
                    for q in range(MACRO):
                        j = m * MACRO + q
                        nc.vector.tensor_scalar_add(
                            sg_sb[:, j * BLK : (j + 1) * BLK], spb, pQ[:, q : q + 1]
                        )

                out_eng.dma_start(
                    out=mur[n], in_=mu_sb.rearrange("p (j c) -> p j c", c=BLK)
                )
                out_eng.dma_start(
                    out=sgr[n], in_=sg_sb.rearrange("p (j c) -> p j c", c=BLK)
                )


def _host_consts(w_mu, w_sigma, b_mu, b_sigma):
    f32 = np.float32
    w_mu = np.ascontiguousarray(w_mu, dtype=f32)
    sp_w = np.logaddexp(w_sigma.astype(f32), f32(0.0)).astype(f32)
    sp_b = np.logaddexp(b_sigma.astype(f32), f32(0.0)).astype(f32)
    consts = {
        "wmu": w_mu,
        "ident": np.eye(D, dtype=f32),
        "bmu4": np.ascontiguousarray(
            np.broadcast_to(b_mu.astype(f32), (D, MACRO, D)).reshape(D, MACRO * D)
        ),
        "spb": np.ascontiguousarray(np.broadcast_to(sp_b, (D, D))),
        "sqw": np.sqrt(sp_w).reshape(D, 1).astype(f32),
        "ones": np.ones((D, 1), dtype=f32),
    }
    # kl depends only on params; fp32 on host, mirroring the jax ops.
    term1 = f32(D) * np.log(sp_w)
    term2 = np.sum(np.abs(w_mu), dtype=f32)
    term3 = f32(D) * sp_w
    kl = f32(-0.5) * np.mean(term1 - term2 - term3, dtype=f32)
    return consts, np.float32(kl)


_NC_CACHE = {}


def _get_nc(rows):
    if rows not in _NC_CACHE:
        _NC_CACHE[rows] = _build(rows)
    return _NC_CACHE[rows]


def run_on_cores(inputs, consts, trace=False):
    from concourse.bass_utils import run_bass_kernel_spmd

    nc = _get_nc(ROWS)
    x = np.ascontiguousarray(inputs, dtype=np.float32)
    in_maps = [
        {"x": x[i * ROWS : (i + 1) * ROWS], **consts} for i in range(NCORES)
    ]
    res = run_bass_kernel_spmd(nc, in_maps, list(range(NCORES)), trace=trace)
    mu = np.concatenate([r["mu"] for r in res.results], axis=0)
    sg = np.concatenate([r["sigma"] for r in res.results], axis=0)
    return mu, sg, res


def kernel(inputs, w_mu, w_sigma, b_mu, b_sigma):
    consts, kl = _host_consts(w_mu, w_sigma, b_mu, b_sigma)
    mu, sg, _ = run_on_cores(inputs, consts, trace=False)
    mu_out = mu.reshape(BATCH, 1, D)
    sigma_out = sg.reshape(BATCH, 1, D)
    return mu_out, sigma_out, kl
